# revision 5
# baseline (speedup 1.0000x reference)
"""Trainium2 Bass kernel for nn_DiffusionTransformerBlock (B=1, N=1024, D=384, H=16, DP=128).

Sharding: query rows (i) split 128/core across 8 NeuronCores; small weights
replicated; each core computes its 128 output rows end-to-end (no collectives).

Pair-bias path (the memory-bound 512 MiB term): pair_cond is host-cast to bf16,
DMA-transposed into [c=128, j] tiles, and LayerNorm is folded algebraically:
  pb[h] = (u[h] - m * s_col[h]) * rsqrt(var + eps),  u = W_eff^T t, m/msq from
a ones column and a squared pass. Raw projections are computed on the PE,
strip-stacked in PSUM, bounced through DRAM to flip [h, j]-strips into
PB[i, h*1024+j], then fixed up with broadcast tensor ops.

Attention/FFN: activations kept transposed [d, token]; heads padded 24->32 so
all PE strips are 32-aligned; pair bias added to logits via identity-matmul.
"""
import sys

sys.path.insert(0, "/opt/trn_rl_repo")

import numpy as np
import ml_dtypes
from contextlib import ExitStack

from concourse import bacc, mybir
import concourse.tile as tile
from concourse.bass_utils import run_bass_kernel_spmd

BF16 = ml_dtypes.bfloat16
F32 = mybir.dt.float32
BF = mybir.dt.bfloat16
AF = mybir.ActivationFunctionType
OP = mybir.AluOpType

N, D, DP, H = 1024, 384, 128, 16
DH = D // H            # 24
D2 = 512               # padded qkv width (16 heads x 32)
DF = 4 * D             # 1536
NI = 128               # query rows per core
NCORES = 8
EPS = 1e-5

_CACHE = {}


def _build(apply_mask: bool):
    nc = bacc.Bacc("TRN2", target_bir_lowering=False)

    inp = {}

    def din(name, shape, dt):
        inp[name] = nc.dram_tensor(name, shape, dt, kind="ExternalInput")
        return inp[name]

    pair = din("pair", [NI * N, DP], BF)
    x_full = din("x_full", [N, D], F32)
    sc_full = din("sc_full", [N, D], F32)
    xrows_d = din("xrows", [NI, D], F32)
    scrows_d = din("scrows", [NI, D], F32)
    w_aug = din("w_aug", [DP, 17], BF)
    nscol = din("nscol", [128, H], F32)
    ident = din("ident", [128, 128], BF)
    # 384-row weights chunked to [128, 3*X]; 512-row to [128, 4*X]; 1536-row to [128, 12*X]
    a_sc_w = din("a_sc_w", [128, 3 * D], BF)
    a_sh_w = din("a_sh_w", [128, 3 * D], BF)
    a_sc_b = din("a_sc_b", [128, 3], F32)
    wq2 = din("wq2", [128, 3 * D2], BF)
    bq2 = din("bq2", [128, 4], F32)
    wk2 = din("wk2", [128, 3 * D2], BF)
    wv2 = din("wv2", [128, 3 * D2], BF)
    wg2 = din("wg2", [128, 3 * D2], BF)
    wo2 = din("wo2", [128, 4 * D], BF)
    f_sc_w = din("f_sc_w", [128, 3 * D], BF)
    f_sh_w = din("f_sh_w", [128, 3 * D], BF)
    f_sc_b = din("f_sc_b", [128, 3], F32)
    w1 = din("w1", [128, 3 * DF], BF)
    w2 = din("w2", [128, 3 * DF], BF)
    w3 = din("w3", [128, 12 * D], BF)
    wgate = din("wgate", [128, 3 * D], BF)
    if apply_mask:
        maskrep = din("maskrep", [128, N], F32)

    out_d = nc.dram_tensor("out", [NI, D], F32, kind="ExternalOutput")

    # internal DRAM bounce buffers for the pair-bias partition shuffle
    pb_dram = nc.dram_tensor("pb_dram", [NI, H * N], BF, kind="Internal")
    mq_dram = nc.dram_tensor("mq_dram", [NI, 2 * N], BF, kind="Internal")

    with ExitStack() as ctx:
        tc = ctx.enter_context(tile.TileContext(nc))

        wp = ctx.enter_context(tc.tile_pool(name="wp", bufs=1))
        actp = ctx.enter_context(tc.tile_pool(name="actp", bufs=1))
        smalls = ctx.enter_context(tc.tile_pool(name="smalls", bufs=4))

        W = {}
        for name, t in inp.items():
            if name in ("pair", "x_full", "sc_full", "xrows", "scrows"):
                continue
            w = wp.tile(list(t.shape), t.dtype, tag=name)
            nc.scalar.dma_start(out=w, in_=t[:, :])
            W[name] = w

        eps_t = smalls.tile([128, 1], F32, tag="eps", name="eps")
        nc.vector.memset(eps_t, EPS)

        # persistent activations
        a_T = [actp.tile([128, N], BF, tag=f"a_T{c}", name=f"a_T{c}") for c in range(3)]
        k_T2 = [actp.tile([128, N], BF, tag=f"k_T2{c}", name=f"k_T2{c}") for c in range(4)]
        v2 = [actp.tile([128, D2], BF, tag=f"v2_{t}", name=f"v2_{t}") for t in range(8)]
        q_T2 = [actp.tile([128, 128], BF, tag=f"q_T2{c}", name=f"q_T2{c}") for c in range(4)]
        g_T2 = [actp.tile([128, 128], BF, tag=f"g_T2{c}", name=f"g_T2{c}") for c in range(4)]
        ffg = actp.tile([128, D], F32, tag="ffg", name="ffg")
        xr_f = actp.tile([128, D], F32, tag="xr_f", name="xr_f")
        PB = actp.tile([128, H * N], BF, tag="PB", name="PB")
        mq_sb = actp.tile([128, 2 * N], BF, tag="mq_sb", name="mq_sb")
        sums = actp.tile([128, H], F32, tag="sums", name="sums")

        def ln_normalize(pool, src_ap, dst_tile):
            """LayerNorm over free dim (384) -> dst (bf16)."""
            st6 = smalls.tile([128, 6], F32, tag="st6", name="st6")
            nc.vector.bn_stats(out=st6, in_=src_ap)
            mv = smalls.tile([128, 2], F32, tag="mv", name="mv")
            nc.vector.bn_aggr(out=mv, in_=st6)
            std = smalls.tile([128, 1], F32, tag="std", name="std")
            nc.scalar.activation(out=std, in_=mv[:, 1:2], func=AF.Sqrt, bias=eps_t, scale=1.0)
            rstd = smalls.tile([128, 1], F32, tag="rstd", name="rstd")
            nc.vector.reciprocal(out=rstd, in_=std)
            negmr = smalls.tile([128, 1], F32, tag="negmr", name="negmr")
            nc.vector.tensor_scalar(out=negmr, in0=mv[:, 0:1], scalar1=rstd, scalar2=-1.0,
                                    op0=OP.mult, op1=OP.mult)
            nc.vector.tensor_scalar(out=dst_tile, in0=src_ap, scalar1=rstd, scalar2=negmr,
                                    op0=OP.mult, op1=OP.add)

        # =====================================================================
        # PREP PHASE: LN, transposes, a, k, v, q, g, FFN
        # =====================================================================
        with tc.tile_pool(name="prep", bufs=1) as prep, \
             tc.tile_pool(name="prep2", bufs=2) as prep2, \
             tc.tile_pool(name="mmps", bufs=3, space="PSUM") as mmps, \
             tc.tile_pool(name="trps", bufs=2, space="PSUM") as trps:

            s_n = []
            xln_n = []
            for t in range(8):
                xt = prep.tile([128, D], F32, tag=f"xt{t}", name=f"xt{t}")
                nc.sync.dma_start(out=xt, in_=x_full[128 * t:128 * (t + 1), :])
                xl = prep.tile([128, D], BF, tag=f"xl{t}", name=f"xl{t}")
                ln_normalize(prep, xt, xl)
                xln_n.append(xl)
                st = prep.tile([128, D], F32, tag=f"st{t}", name=f"st{t}")
                nc.sync.dma_start(out=st, in_=sc_full[128 * t:128 * (t + 1), :])
                sl = prep.tile([128, D], BF, tag=f"sl{t}", name=f"sl{t}")
                ln_normalize(prep, st, sl)
                s_n.append(sl)

            # transpose to [d, token]
            s_T = [prep.tile([128, N], BF, tag=f"s_T{c}", name=f"s_T{c}") for c in range(3)]
            xln_T = [prep.tile([128, N], BF, tag=f"xln_T{c}", name=f"xln_T{c}") for c in range(3)]
            for c in range(3):
                for src, dstl in ((s_n, s_T), (xln_n, xln_T)):
                    trp = trps.tile([128, N], BF, tag="tr", name="tr")
                    for t in range(8):
                        nc.tensor.transpose(trp[:, 128 * t:128 * (t + 1)],
                                            src[t][:, 128 * c:128 * (c + 1)], W["ident"])
                    nc.scalar.copy(dstl[c], trp)

            # rows-only LN + transposes (core's own 128 rows)
            nc.sync.dma_start(out=xr_f, in_=xrows_d[:, :])
            sr_f = prep.tile([128, D], F32, tag="sr_f", name="sr_f")
            nc.sync.dma_start(out=sr_f, in_=scrows_d[:, :])
            xlr = prep.tile([128, D], BF, tag="xlr", name="xlr")
            ln_normalize(prep, xr_f, xlr)
            slr = prep.tile([128, D], BF, tag="slr", name="slr")
            ln_normalize(prep, sr_f, slr)
            srows_T = [prep.tile([128, 128], BF, tag=f"srT{c}", name=f"srT{c}") for c in range(3)]
            xlnrows_T = [prep.tile([128, 128], BF, tag=f"xlrT{c}", name=f"xlrT{c}") for c in range(3)]
            trp = trps.tile([128, N], BF, tag="tr", name="tr")
            for c in range(3):
                nc.tensor.transpose(trp[:, 128 * c:128 * (c + 1)],
                                    slr[:, 128 * c:128 * (c + 1)], W["ident"])
                nc.tensor.transpose(trp[:, 384 + 128 * c:384 + 128 * (c + 1)],
                                    xlr[:, 128 * c:128 * (c + 1)], W["ident"])
            for c in range(3):
                nc.vector.tensor_copy(srows_T[c], trp[:, 128 * c:128 * (c + 1)])
                nc.vector.tensor_copy(xlnrows_T[c], trp[:, 384 + 128 * c:384 + 128 * (c + 1)])

            def adaln_T(scw, shw, scb, s_src, xln_src, dst, width):
                """dst[e][:, j] = sigmoid(scw^T s + b) * xln + shw^T s, transposed layout.

                s_src/xln_src: list of 3 dc-chunk tiles [128, width].
                dst: list of 3 e-chunk tiles [128, width]."""
                nh = width // 512 if width >= 512 else 1
                hw = width // nh
                for e in range(3):
                    for hf in range(nh):
                        sl = slice(hw * hf, hw * (hf + 1))
                        ps = mmps.tile([128, 512], F32, tag="mm", name="mm")
                        for dc in range(3):
                            nc.tensor.matmul(ps[:, 0:hw], lhsT=W[scw][:, D * dc + 128 * e:D * dc + 128 * e + 128],
                                             rhs=s_src[dc][:, sl], start=(dc == 0), stop=(dc == 2))
                        sg = prep2.tile([128, 512], BF, tag="adaln_sg", name="adaln_sg")
                        nc.scalar.activation(out=sg[:, 0:hw], in_=ps[:, 0:hw], func=AF.Sigmoid,
                                             bias=W[scb][:, e:e + 1], scale=1.0)
                        ps2 = mmps.tile([128, 512], F32, tag="mm", name="mm")
                        for dc in range(3):
                            nc.tensor.matmul(ps2[:, 0:hw], lhsT=W[shw][:, D * dc + 128 * e:D * dc + 128 * e + 128],
                                             rhs=s_src[dc][:, sl], start=(dc == 0), stop=(dc == 2))
                        t1 = prep2.tile([128, 512], BF, tag="adaln_t1", name="adaln_t1")
                        nc.vector.tensor_tensor(out=t1[:, 0:hw], in0=sg[:, 0:hw],
                                                in1=xln_src[e][:, sl], op=OP.mult)
                        nc.vector.tensor_tensor(out=dst[e][:, sl], in0=t1[:, 0:hw],
                                                in1=ps2[:, 0:hw], op=OP.add)

            # a (all tokens, for k/v) and a-rows / f-rows (core rows)
            adaln_T("a_sc_w", "a_sh_w", "a_sc_b", s_T, xln_T, a_T, N)
            arows_T = [prep.tile([128, 128], BF, tag=f"arT{c}", name=f"arT{c}") for c in range(3)]
            frows_T = [prep.tile([128, 128], BF, tag=f"frT{c}", name=f"frT{c}") for c in range(3)]
            adaln_T("a_sc_w", "a_sh_w", "a_sc_b", srows_T, xlnrows_T, arows_T, 128)
            adaln_T("f_sc_w", "f_sh_w", "f_sc_b", srows_T, xlnrows_T, frows_T, 128)

            # k_T2 [e'-chunk 4][128, N]
            for e in range(4):
                for hf in range(2):
                    sl = slice(512 * hf, 512 * (hf + 1))
                    ps = mmps.tile([128, 512], F32, tag="mm", name="mm")
                    for dc in range(3):
                        nc.tensor.matmul(ps, lhsT=W["wk2"][:, D2 * dc + 128 * e:D2 * dc + 128 * e + 128],
                                         rhs=a_T[dc][:, sl], start=(dc == 0), stop=(dc == 2))
                    nc.scalar.copy(k_T2[e][:, sl], ps)

            # v2 [tok 8][128, D2] (natural layout)
            for t in range(8):
                ps = mmps.tile([128, 512], F32, tag="mm", name="mm")
                for dc in range(3):
                    nc.tensor.matmul(ps, lhsT=a_T[dc][:, 128 * t:128 * (t + 1)],
                                     rhs=W["wv2"][:, D2 * dc:D2 * (dc + 1)],
                                     start=(dc == 0), stop=(dc == 2))
                nc.vector.tensor_copy(v2[t], ps)

            # q_T2 / g_T2 (core rows only)
            for e in range(4):
                ps = mmps.tile([128, 512], F32, tag="mm", name="mm")
                for dc in range(3):
                    nc.tensor.matmul(ps[:, 0:128], lhsT=W["wq2"][:, D2 * dc + 128 * e:D2 * dc + 128 * e + 128],
                                     rhs=arows_T[dc], start=(dc == 0), stop=(dc == 2))
                nc.scalar.add(q_T2[e], ps[:, 0:128], add=W["bq2"][:, e:e + 1])
                ps2 = mmps.tile([128, 512], F32, tag="mm", name="mm")
                for dc in range(3):
                    nc.tensor.matmul(ps2[:, 0:128], lhsT=W["wg2"][:, D2 * dc + 128 * e:D2 * dc + 128 * e + 128],
                                     rhs=arows_T[dc], start=(dc == 0), stop=(dc == 2))
                nc.scalar.activation(out=g_T2[e], in_=ps2[:, 0:128], func=AF.Sigmoid)

            # FFN: hdn = silu(f@w1) * (f@w2); ffg = sigmoid(s@wgate) * (hdn@w3)
            hdn_T = [prep.tile([128, 128], BF, tag=f"hdn{d}", name=f"hdn{d}") for d in range(12)]
            for d in range(12):
                ps1 = mmps.tile([128, 512], F32, tag="mm", name="mm")
                for dc in range(3):
                    nc.tensor.matmul(ps1[:, 0:128], lhsT=W["w1"][:, DF * dc + 128 * d:DF * dc + 128 * d + 128],
                                     rhs=frows_T[dc], start=(dc == 0), stop=(dc == 2))
                ps2 = mmps.tile([128, 512], F32, tag="mm", name="mm")
                for dc in range(3):
                    nc.tensor.matmul(ps2[:, 0:128], lhsT=W["w2"][:, DF * dc + 128 * d:DF * dc + 128 * d + 128],
                                     rhs=frows_T[dc], start=(dc == 0), stop=(dc == 2))
                sg1 = prep2.tile([128, 128], BF, tag="ffn_sg", name="ffn_sg")
                nc.scalar.activation(out=sg1, in_=ps1[:, 0:128], func=AF.Sigmoid)
                sil = prep2.tile([128, 128], BF, tag="ffn_sil", name="ffn_sil")
                nc.vector.tensor_tensor(out=sil, in0=ps1[:, 0:128], in1=sg1, op=OP.mult)
                nc.vector.tensor_tensor(out=hdn_T[d], in0=sil, in1=ps2[:, 0:128], op=OP.mult)

            psf = mmps.tile([128, 512], F32, tag="ffo", name="ffo")
            for d in range(12):
                nc.tensor.matmul(psf[:, 0:D], lhsT=hdn_T[d], rhs=W["w3"][:, D * d:D * (d + 1)],
                                 start=(d == 0), stop=(d == 11))
            psg = mmps.tile([128, 512], F32, tag="ffo", name="ffo")
            for dc in range(3):
                nc.tensor.matmul(psg[:, 0:D], lhsT=srows_T[dc], rhs=W["wgate"][:, D * dc:D * (dc + 1)],
                                 start=(dc == 0), stop=(dc == 2))
            sgf = prep2.tile([128, D], BF, tag="ffn_gate", name="ffn_gate")
            nc.scalar.activation(out=sgf, in_=psg[:, 0:D], func=AF.Sigmoid)
            nc.vector.tensor_tensor(out=ffg, in0=psf[:, 0:D], in1=sgf, op=OP.mult)

        # =====================================================================
        # PAIR PHASE: 32 groups of 4 i-rows
        # =====================================================================
        with tc.tile_pool(name="pairp", bufs=2) as pairp, \
             tc.tile_pool(name="sqp", bufs=2) as sqp, \
             tc.tile_pool(name="stgp", bufs=2) as stgp, \
             tc.tile_pool(name="uTps", bufs=2, space="PSUM") as uTps, \
             tc.tile_pool(name="u2ps", bufs=2, space="PSUM") as u2ps:

            for blk in range(32):
                tp = pairp.tile([128, 4 * N], BF, tag="tp", name="tp")
                nc.sync.dma_start(out=tp, in_=pair[4 * N * blk:4 * N * (blk + 1), :],
                                  transpose=True)
                uT = uTps.tile([128, N], F32, tag="uT", name="uT")
                u2 = u2ps.tile([128, N], F32, tag="u2", name="u2")
                for s in range(4):
                    io = N * s
                    sq = sqp.tile([128, N], BF, tag="sq", name="sq")
                    if (blk * 4 + s) % 2 == 0:
                        nc.vector.tensor_tensor(out=sq, in0=tp[:, io:io + N],
                                                in1=tp[:, io:io + N], op=OP.mult)
                    else:
                        nc.scalar.activation(out=sq, in_=tp[:, io:io + N], func=AF.Square)
                    for hf in range(2):
                        sl = slice(512 * hf, 512 * (hf + 1))
                        nc.tensor.matmul(uT[32 * s:32 * s + 17, sl], lhsT=W["w_aug"],
                                         rhs=tp[:, io + 512 * hf:io + 512 * (hf + 1)],
                                         start=True, stop=True, tile_position=(0, 32 * s))
                        nc.tensor.matmul(u2[32 * s:32 * s + 17, sl], lhsT=W["w_aug"],
                                         rhs=sq[:, sl],
                                         start=True, stop=True, tile_position=(0, 32 * s))
                stg = stgp.tile([128, 2 * N], BF, tag="stg", name="stg")
                nc.vector.tensor_copy(stg[:, 0:N], uT)
                nc.scalar.copy(stg[:, N:2 * N], u2)
                for s in range(4):
                    r = blk * 4 + s
                    nc.scalar.dma_start(
                        out=pb_dram[r:r + 1, :].rearrange("o (h j) -> (o h) j", h=H),
                        in_=stg[32 * s:32 * s + 16, 0:N])
                    nc.scalar.dma_start(out=mq_dram[r:r + 1, :],
                                        in_=stg[32 * s + 16:32 * s + 17, :])

        # =====================================================================
        # FIXUP: PB = u*r - (m*r)*s_col  (+ mask bias)
        # =====================================================================
        with tc.tile_pool(name="fix", bufs=1) as fix:
            nc.sync.dma_start(out=PB, in_=pb_dram[:, :])
            nc.sync.dma_start(out=mq_sb, in_=mq_dram[:, :])
            m_ap = mq_sb[:, 0:N]
            msq_ap = mq_sb[:, N:2 * N]
            m2 = fix.tile([128, N], F32, tag="m2", name="m2")
            nc.vector.tensor_tensor(out=m2, in0=m_ap, in1=m_ap, op=OP.mult)
            var = fix.tile([128, N], F32, tag="var", name="var")
            nc.vector.tensor_tensor(out=var, in0=msq_ap, in1=m2, op=OP.subtract)
            stdv = fix.tile([128, N], F32, tag="stdv", name="stdv")
            nc.scalar.activation(out=stdv, in_=var, func=AF.Sqrt, bias=eps_t, scale=1.0)
            R = fix.tile([128, N], F32, tag="R", name="R")
            nc.vector.reciprocal(out=R, in_=stdv)
            MR = fix.tile([128, N], F32, tag="MR", name="MR")
            nc.vector.tensor_tensor(out=MR, in0=m_ap, in1=R, op=OP.mult)

            PB3 = PB[:, :].rearrange("p (h j) -> p h j", h=H)
            R_b = R[:, :].unsqueeze(1).broadcast_to([128, H, N])
            MR_b = MR[:, :].unsqueeze(1).broadcast_to([128, H, N])
            NS_b = W["nscol"][:, :].unsqueeze(2).broadcast_to([128, H, N])
            tmp16 = fix.tile([128, H * N], BF, tag="tmp16", name="tmp16")
            tmp3 = tmp16[:, :].rearrange("p (h j) -> p h j", h=H)
            nc.vector.tensor_tensor(out=tmp3, in0=MR_b, in1=NS_b, op=OP.mult)
            nc.vector.tensor_tensor(out=PB3, in0=PB3, in1=R_b, op=OP.mult)
            nc.vector.tensor_tensor(out=PB3, in0=PB3, in1=tmp3, op=OP.add)
            if apply_mask:
                MK_b = W["maskrep"][:, :].unsqueeze(1).broadcast_to([128, H, N])
                nc.vector.tensor_tensor(out=PB3, in0=PB3, in1=MK_b, op=OP.add)

        # =====================================================================
        # ATTENTION PHASE
        # =====================================================================
        with tc.tile_pool(name="soft", bufs=2) as soft, \
             tc.tile_pool(name="lgps", bufs=2, space="PSUM") as lgps, \
             tc.tile_pool(name="ogps", bufs=2, space="PSUM") as ogps, \
             tc.tile_pool(name="atps", bufs=1, space="PSUM") as atps, \
             tc.tile_pool(name="trps2", bufs=1, space="PSUM") as trps2:

            att_ps = atps.tile([128, D], F32, tag="att", name="att")
            og = None
            for h in range(H):
                chunk, sub = h // 4, h % 4
                lg = lgps.tile([128, N], F32, tag="lg", name="lg")
                for hf in range(2):
                    sl = slice(512 * hf, 512 * (hf + 1))
                    nc.tensor.matmul(lg[:, sl],
                                     lhsT=q_T2[chunk][32 * sub:32 * sub + 32, :],
                                     rhs=k_T2[chunk][32 * sub:32 * sub + 32, sl],
                                     start=True, stop=False, tile_position=(32 * sub, 0))
                    nc.tensor.matmul(lg[:, sl], lhsT=W["ident"],
                                     rhs=PB[:, N * h + 512 * hf:N * h + 512 * (hf + 1)],
                                     start=False, stop=True, tile_position=(0, 0))
                P = soft.tile([128, N], BF, tag="P", name="P")
                nc.scalar.activation(out=P, in_=lg, func=AF.Exp)
                nc.vector.reduce_sum(sums[:, h:h + 1], P, axis=mybir.AxisListType.X)
                rs = smalls.tile([128, 1], F32, tag="rs", name="rs")
                nc.vector.reciprocal(out=rs, in_=sums[:, h:h + 1])
                Pn = soft.tile([128, N], BF, tag="Pn", name="Pn")
                nc.vector.tensor_scalar_mul(Pn, P, rs)
                trp = trps2.tile([128, N], BF, tag="ptr", name="ptr")
                for jb in range(8):
                    nc.tensor.transpose(trp[:, 128 * jb:128 * (jb + 1)],
                                        Pn[:, 128 * jb:128 * (jb + 1)], W["ident"])
                PT = soft.tile([128, N], BF, tag="PT", name="PT")
                nc.scalar.copy(PT, trp)
                if sub == 0:
                    og = ogps.tile([128, 128], F32, tag="og", name="og")
                for jb in range(8):
                    nc.tensor.matmul(og[32 * sub:32 * sub + 32, :],
                                     lhsT=v2[jb][:, 32 * h:32 * h + 32],
                                     rhs=PT[:, 128 * jb:128 * (jb + 1)],
                                     start=(jb == 0), stop=(jb == 7),
                                     tile_position=(0, 32 * sub))
                if sub == 3:
                    go = soft.tile([128, 128], BF, tag="go", name="go")
                    nc.vector.tensor_tensor(out=go, in0=g_T2[chunk], in1=og, op=OP.mult)
                    nc.tensor.matmul(att_ps, lhsT=go, rhs=W["wo2"][:, D * chunk:D * (chunk + 1)],
                                     start=(chunk == 0), stop=(chunk == 3))

            # final: out = xrows + attn_out + ff_out
            of1 = soft.tile([128, D], F32, tag="of1", name="of1")
            nc.vector.tensor_tensor(out=of1, in0=xr_f, in1=att_ps, op=OP.add)
            of2 = soft.tile([128, D], F32, tag="of2", name="of2")
            nc.vector.tensor_tensor(out=of2, in0=of1, in1=ffg, op=OP.add)
            nc.sync.dma_start(out=out_d[:, :], in_=of2)

    nc.compile()
    return nc


def _get_nc(apply_mask: bool):
    if apply_mask not in _CACHE:
        _CACHE[apply_mask] = _build(apply_mask)
    return _CACHE[apply_mask]


def _chunkP(w, p=128):
    """[k*128, X] -> [128, k*X] with chunk c at cols [c*X, (c+1)*X)."""
    k = w.shape[0] // p
    return np.ascontiguousarray(
        w.reshape(k, p, w.shape[1]).transpose(1, 0, 2).reshape(p, k * w.shape[1]))


def _pad_heads(w, scale=1.0):
    """[D, H*24] -> [D, H*32], scaled."""
    out = np.zeros((w.shape[0], H * 32), np.float32)
    out.reshape(w.shape[0], H, 32)[:, :, :DH] = w.reshape(w.shape[0], H, DH) * scale
    return out


def _make_in_maps(inputs):
    x = np.asarray(inputs["x"], np.float32)            # [1, N, D]
    sc = np.asarray(inputs["single_cond"], np.float32)
    pc = np.asarray(inputs["pair_cond"], np.float32)   # [1, N, N, DP]
    mask = np.asarray(inputs["mask"])                  # [1, N] bool

    apply_mask = not bool(mask.all())

    f = lambda k: np.asarray(inputs[k], np.float32)
    scale = 1.0 / np.sqrt(np.float32(DH))

    w_eff = f("pb_ln_w")[:, None] * f("pb_w")          # [128, 16]
    w_aug = np.concatenate([w_eff, np.full((DP, 1), 1.0 / DP, np.float32)], 1).astype(BF16)
    nscol = np.tile(-w_eff.sum(0)[None, :], (128, 1)).astype(np.float32)
    ident = np.eye(128, dtype=np.float32).astype(BF16)

    wq2 = _chunkP(_pad_heads(f("wq"), scale)).astype(BF16)
    bq2p = np.zeros(D2, np.float32)
    bq2p.reshape(H, 32)[:, :DH] = f("bq").reshape(H, DH) * scale
    bq2 = np.ascontiguousarray(bq2p.reshape(4, 128).T)
    wk2 = _chunkP(_pad_heads(f("wk"))).astype(BF16)
    wv2 = _chunkP(_pad_heads(f("wv"))).astype(BF16)
    wg2 = _chunkP(_pad_heads(f("wg"))).astype(BF16)
    wo2p = np.zeros((D2, D), np.float32)
    wo2p.reshape(H, 32, D)[:, :DH, :] = f("wo").reshape(H, DH, D)
    wo2 = _chunkP(wo2p).astype(BF16)

    shared = {
        "x_full": np.ascontiguousarray(x[0]),
        "sc_full": np.ascontiguousarray(sc[0]),
        "w_aug": w_aug, "nscol": nscol, "ident": ident,
        "a_sc_w": _chunkP(f("a_sc_w")).astype(BF16),
        "a_sh_w": _chunkP(f("a_sh_w")).astype(BF16),
        "a_sc_b": np.ascontiguousarray(f("a_sc_b").reshape(3, 128).T),
        "wq2": wq2, "bq2": bq2, "wk2": wk2, "wv2": wv2, "wg2": wg2, "wo2": wo2,
        "f_sc_w": _chunkP(f("f_sc_w")).astype(BF16),
        "f_sh_w": _chunkP(f("f_sh_w")).astype(BF16),
        "f_sc_b": np.ascontiguousarray(f("f_sc_b").reshape(3, 128).T),
        "w1": _chunkP(f("w1")).astype(BF16),
        "w2": _chunkP(f("w2")).astype(BF16),
        "w3": _chunkP(f("w3")).astype(BF16),
        "wgate": _chunkP(f("wgate")).astype(BF16),
    }
    if apply_mask:
        mb = np.where(mask[0], 0.0, -1e9).astype(np.float32)
        shared["maskrep"] = np.tile(mb[None, :], (128, 1))

    pair_bf = pc.reshape(N * N, DP).astype(BF16).reshape(NCORES, NI * N, DP)

    in_maps = []
    for m in range(NCORES):
        im = dict(shared)
        im["pair"] = pair_bf[m]
        im["xrows"] = np.ascontiguousarray(x[0, NI * m:NI * (m + 1)])
        im["scrows"] = np.ascontiguousarray(sc[0, NI * m:NI * (m + 1)])
        in_maps.append(im)

    return in_maps


def kernel(**inputs):
    mask = np.asarray(inputs["mask"])
    apply_mask = not bool(mask.all())
    nc = _get_nc(apply_mask)
    in_maps = _make_in_maps(inputs)
    res = run_bass_kernel_spmd(nc, in_maps, core_ids=list(range(NCORES)))
    out = np.concatenate([res.results[m]["out"] for m in range(NCORES)], axis=0)
    return out[None].astype(np.float32)
